# revision 15
# baseline (speedup 1.0000x reference)
"""Trainium2 Bass kernel for DiagGraphSAGENet (GraphSAGE message passing).

Computes, for node features x [N, 512] and edge list [2, E]:
    agg   = segment_sum(x[src], dst)                      # sum over in-edges
    loc   = clip(agg @ Wl1.T + bl1 + x @ Wr1.T, -100, 100)
    scale = min(softplus(agg @ Wl2.T + bl2 + x @ Wr2.T) + 0.001, 100)

Strategy (8 NeuronCores, SPMD single NEFF):
  - Destination-node sharding: core c owns nodes [c*6250, (c+1)*6250).
  - Host sorts edges by (dst core, 128-node dst block, src-half); device
    gathers source rows with the Q7 dma_gather primitive (int16 indices,
    so x is split at row 32768 into lo/hi gather sources). x is cast to
    bf16 host-side to halve the gather traffic (the dominant DMA term).
  - Segment sum realized as one-hot matmul: for each 128-edge tile, build
    M[edge, dst_local] = (dstloc[edge] == iota) on DVE (bf16), accumulate
    psum_agg[128 nodes, 512] += M.T @ Xe on the tensor engine.
  - agg transposed on-PE; the four 512x512 GEMMs run in bf16 against
    host-pretransposed bf16 weights; biases folded in via a K=1 ones-row
    matmul; clip/softplus epilogue on DVE+ACT (single Softplus table).
  - PSUM->SBUF copies run on the scalar engine to keep DVE free.
  - Each core writes bf16 loc/scale for its 6250 nodes; host reassembles
    and casts to f32.
"""

import math
import numpy as np

# ---------------------------------------------------------------- config

class Cfg:
    def __init__(self, n_nodes=50000, n_edges=800000, d=512, n_cores=8,
                 gather_bf16=True, out_bf16=True, nan_safe_pad=True,
                 gather_chunk_tiles=8, gbufs=4, act_copies=True,
                 use_softplus=False, n_swdge_queues=4):
        self.N = n_nodes
        self.E = n_edges
        self.D = d
        self.C = n_cores
        assert n_nodes % n_cores == 0
        self.NPC = n_nodes // n_cores            # nodes per core
        self.B = math.ceil(self.NPC / 128)       # dst blocks per core
        self.ROWS = self.B * 128                 # padded out rows per core
        self.SPLIT = min(32768, n_nodes)         # int16 gather split point
        self.gather_bf16 = gather_bf16
        self.out_bf16 = out_bf16
        # pad gather slots with idx=0 (real row, avoids NaN garbage in SBUF)
        self.nan_safe_pad = nan_safe_pad
        # max 128-row tiles per dma_gather instruction
        self.gather_chunk_tiles = gather_chunk_tiles
        self.gbufs = gbufs
        self.act_copies = act_copies
        self.use_softplus = use_softplus
        self.n_swdge_queues = n_swdge_queues


CFG = Cfg()

# ---------------------------------------------------------------- host prep

def _prep_edges(cfg, src, dst):
    """Sort/pad edges into per-core per-block gather streams.

    Returns (caps [B,2] shared tile caps, tile_off [B,2], total_tiles,
    idx_grid [C,16,8*total_tiles] int16, dstloc [C,128,total_tiles] f32).
    """
    C, B, NPC, SPLIT = cfg.C, cfg.B, cfg.NPC, cfg.SPLIT
    ecore = dst // NPC
    eblk = (dst % NPC) // 128
    eslot = (dst % NPC) % 128
    ehi = (src >= SPLIT).astype(np.int64)
    key = (ecore * B + eblk) * 2 + ehi
    # secondary sort by src so each segment's gather descriptors sweep HBM
    # monotonically (row-buffer locality on the random gather)
    order = np.lexsort((src, key))
    src_s = src[order]
    eslot_s = eslot[order]
    counts = np.bincount(key, minlength=C * B * 2)
    start = np.concatenate([[0], np.cumsum(counts)])
    cnt = counts.reshape(C, B, 2)
    caps = -(-cnt // 128)                # ceil tiles per (core, blk, stream)
    caps = caps.max(axis=0)              # [B, 2] shared across cores (SPMD)
    tile_off = np.zeros((B, 2), np.int64)
    off = 0
    for b in range(B):
        for s in range(2):
            tile_off[b, s] = off
            off += caps[b, s]
    total_tiles = int(off)

    pad_idx = 0 if cfg.nan_safe_pad else -1
    idx_grid = np.full((C, 16, 8 * total_tiles), pad_idx, np.int16)
    dstloc = np.full((C, 128, total_tiles), -1.0, np.float32)
    for c in range(C):
        for b in range(B):
            for s in (0, 1):
                T = int(caps[b, s])
                if T == 0:
                    continue
                k = (c * B + b) * 2 + s
                n = int(counts[k])
                toff = int(tile_off[b, s])
                buf = np.full(T * 128, pad_idx, np.int64)
                buf[:n] = src_s[start[k]:start[k] + n] - (SPLIT if s else 0)
                dl = np.full(T * 128, -1.0, np.float32)
                dl[:n] = eslot_s[start[k]:start[k] + n]
                idx_grid[c, :, 8 * toff:8 * (toff + T)] = (
                    buf.reshape(-1, 16).T.astype(np.int16))
                dstloc[c, :, toff:toff + T] = dl.reshape(T, 128).T
    return caps, tile_off, total_tiles, idx_grid, dstloc


def _prep_host(cfg, x, edge_index, Wl1, bl1, Wr1, Wl2, bl2, Wr2):
    import ml_dtypes
    bf16 = ml_dtypes.bfloat16
    src = np.asarray(edge_index[0]).astype(np.int64)
    dst = np.asarray(edge_index[1]).astype(np.int64)
    x = np.asarray(x, dtype=np.float32)
    caps, tile_off, total_tiles, idx_grid, dstloc = _prep_edges(cfg, src, dst)

    # per-core transposed own features, padded to ROWS columns (bf16)
    xt = np.zeros((cfg.C, cfg.D, cfg.ROWS), bf16)
    for c in range(cfg.C):
        own = x[c * cfg.NPC:(c + 1) * cfg.NPC]
        xt[c][:, :cfg.NPC] = own.T.astype(bf16)

    # weights packed as [128, 16*D]: for w in (Wl1, Wr1, Wl2, Wr2), chunks
    # c of W.T: rows c*128..c*128+127 -> columns (w*4+c)*D .. +D
    kc = cfg.D // 128
    packs = []
    for W in (Wl1, Wr1, Wl2, Wr2):
        WT = np.asarray(W, np.float32).T                     # [D_in, D_out]
        packs.append(WT.reshape(kc, 128, cfg.D).transpose(1, 0, 2)
                     .reshape(128, kc * cfg.D))
    wts = np.concatenate(packs, axis=1).astype(bf16)         # [128, 4*kc*D]

    bias = np.concatenate([np.asarray(bl1, np.float32),
                           np.asarray(bl2, np.float32)])[None, :].astype(bf16)
    iota = np.tile(np.arange(128, dtype=np.float32),
                   (128, 1)).astype(bf16)                    # [128,128]
    iden = np.eye(128, dtype=np.float32).astype(bf16)
    ones = np.ones((1, 128), bf16)

    xg = x.astype(bf16) if cfg.gather_bf16 else x
    dl16 = dstloc.astype(bf16)
    in_maps = []
    for c in range(cfg.C):
        in_maps.append({
            "x": xg,
            "idx": np.tile(idx_grid[c], (8, 1)),
            "dstloc": dl16[c],
            "xt": xt[c],
            "wts": wts,
            "bias": bias,
            "iota": iota,
            "iden": iden,
            "ones": ones,
        })
    return caps, total_tiles, in_maps


# ---------------------------------------------------------------- device

def _build_program(cfg, caps, total_tiles):
    import concourse.bacc as bacc
    import concourse.mybir as mybir
    import concourse.tile as tile

    f32 = mybir.dt.float32
    bf16 = mybir.dt.bfloat16
    gdt = bf16 if cfg.gather_bf16 else mybir.dt.float32r
    odt = bf16 if cfg.out_bf16 else f32
    D, B, SPLIT, N = cfg.D, cfg.B, cfg.SPLIT, cfg.N
    kc = D // 128

    nc = bacc.Bacc("TRN2", target_bir_lowering=False, debug=False,
                   num_swdge_queues=cfg.n_swdge_queues,
                   dynamic_dma_scratch_size=32768)
    x_d = nc.dram_tensor("x", [N, D], gdt, kind="ExternalInput")
    idx_d = nc.dram_tensor("idx", [128, 8 * total_tiles], mybir.dt.int16,
                           kind="ExternalInput")
    dstloc_d = nc.dram_tensor("dstloc", [128, total_tiles], bf16,
                              kind="ExternalInput")
    xt_d = nc.dram_tensor("xt", [D, cfg.ROWS], bf16, kind="ExternalInput")
    wts_d = nc.dram_tensor("wts", [128, 4 * kc * D], bf16,
                           kind="ExternalInput")
    bias_d = nc.dram_tensor("bias", [1, 2 * D], bf16, kind="ExternalInput")
    iota_d = nc.dram_tensor("iota", [128, 128], bf16, kind="ExternalInput")
    iden_d = nc.dram_tensor("iden", [128, 128], bf16, kind="ExternalInput")
    ones_d = nc.dram_tensor("ones", [1, 128], bf16, kind="ExternalInput")
    loc_d = nc.dram_tensor("loc", [cfg.ROWS, D], odt, kind="ExternalOutput")
    scale_d = nc.dram_tensor("scale", [cfg.ROWS, D], odt,
                             kind="ExternalOutput")

    Tmax = int((caps[:, 0] + caps[:, 1]).max())

    with tile.TileContext(nc) as tc:
        with (
            tc.tile_pool(name="const", bufs=1) as constp,
            tc.tile_pool(name="gbuf", bufs=cfg.gbufs) as gpool,
            tc.tile_pool(name="work", bufs=3) as wpool,
            tc.tile_pool(name="mbuf", bufs=4) as mpool,
            tc.tile_pool(name="psum", bufs=2, space="PSUM") as pp,
        ):
            idx_s = constp.tile([128, 8 * total_tiles], mybir.dt.int16)
            nc.sync.dma_start(idx_s[:], idx_d[:])
            dstloc_s = constp.tile([128, total_tiles], bf16)
            nc.sync.dma_start(dstloc_s[:], dstloc_d[:])
            wts_s = constp.tile([128, 4 * kc * D], bf16)
            nc.sync.dma_start(wts_s[:], wts_d[:])
            bias_s = constp.tile([1, 2 * D], bf16)
            nc.sync.dma_start(bias_s[:], bias_d[:])
            iota_s = constp.tile([128, 128], bf16)
            nc.sync.dma_start(iota_s[:], iota_d[:])
            ident_s = constp.tile([128, 128], bf16)
            nc.sync.dma_start(ident_s[:], iden_d[:])
            ones_s = constp.tile([1, 128], bf16)
            nc.sync.dma_start(ones_s[:], ones_d[:])

            if not cfg.nan_safe_pad:
                # pad slots use idx=-1: the gather ucode trims trailing
                # negative idxs, leaving stale SBUF data in those slots.
                # M zeroes their contribution, but 0*NaN=NaN -- so zero the
                # gather ring slots once to guarantee finite stale data.
                for _ in range(cfg.gbufs):
                    gx0 = gpool.tile([128, Tmax * D], gdt, tag="gx")
                    nc.vector.memset(gx0[:], 0.0)

            gq = 0  # round-robin SWDGE queue -> Q7 core pair (2q, 2q+1)
            for b in range(B):
                Tlo, Thi = int(caps[b, 0]), int(caps[b, 1])
                Tb = Tlo + Thi
                toff = int(np.sum(caps[:b]))  # tiles before block b
                # ---- gather source rows for this block's edges
                if Tb > 0:
                    gx = gpool.tile([128, Tmax * D], gdt, tag="gx")
                    GC = cfg.gather_chunk_tiles
                    for seg_T, seg_src, seg_t0, dst_t0 in (
                            (Tlo, x_d[0:SPLIT, :], toff, 0),
                            (Thi, x_d[SPLIT:N, :], toff + Tlo, Tlo)):
                        for t0 in range(0, seg_T, GC):
                            tn = min(GC, seg_T - t0)
                            nc.gpsimd.dma_gather(
                                out_ap=gx[:, (dst_t0 + t0) * D:
                                          (dst_t0 + t0 + tn) * D].rearrange(
                                    "p (t e) -> p t e", e=D),
                                in_ap=seg_src,
                                idxs_ap=idx_s[:, 8 * (seg_t0 + t0):
                                              8 * (seg_t0 + t0 + tn)],
                                num_idxs=tn * 128, num_idxs_reg=tn * 128,
                                elem_size=D,
                                queue_num=gq % cfg.n_swdge_queues)
                            gq += 1
                # ---- own features (transposed) for this block
                xt_s = wpool.tile([128, kc, 128], bf16, tag="xt")
                nc.sync.dma_start(
                    xt_s[:],
                    xt_d[:, b * 128:(b + 1) * 128].rearrange(
                        "(c p) n -> p c n", p=128))
                # ---- aggregation: psum_agg[node, feat] += M.T @ Xe
                agg_s = wpool.tile([128, D], bf16, tag="aggs")
                if Tb > 0:
                    ps_agg = pp.tile([128, D], f32, tag="agg")
                    for t in range(Tb):
                        m = mpool.tile([128, 128], gdt, tag="m")
                        nc.vector.tensor_tensor(
                            out=m[:],
                            in0=dstloc_s[:, toff + t:toff + t + 1]
                                .to_broadcast([128, 128]),
                            in1=iota_s[:],
                            op=mybir.AluOpType.is_equal)
                        nc.tensor.matmul(
                            ps_agg[:], lhsT=m[:],
                            rhs=gx[:, t * D:(t + 1) * D],
                            start=(t == 0), stop=(t == Tb - 1))
                    if cfg.act_copies:
                        nc.scalar.activation(
                            agg_s[:], ps_agg[:],
                            mybir.ActivationFunctionType.Copy)
                    else:
                        nc.vector.tensor_copy(agg_s[:], ps_agg[:])
                else:
                    nc.vector.memset(agg_s[:], 0.0)
                # ---- transpose agg -> aggT (feat-major for GEMM lhsT)
                ps_t = pp.tile([128, D], bf16, tag="aggT")
                for ch in range(kc):
                    nc.tensor.transpose(
                        ps_t[:, ch * 128:(ch + 1) * 128],
                        agg_s[:, ch * 128:(ch + 1) * 128],
                        ident_s[:])
                aggT_s = wpool.tile([128, D], bf16, tag="aggTs")
                if cfg.act_copies:
                    nc.scalar.activation(
                        aggT_s[:], ps_t[:],
                        mybir.ActivationFunctionType.Copy)
                else:
                    nc.vector.tensor_copy(aggT_s[:], ps_t[:])
                # ---- GEMMs: loc / scale heads
                ps_loc = pp.tile([128, D], f32, tag="loc")
                ps_scl = pp.tile([128, D], f32, tag="scl")
                for ps, wbase, bcol in ((ps_loc, 0, 0), (ps_scl, 2, D)):
                    for ch in range(kc):
                        nc.tensor.matmul(
                            ps[:],
                            lhsT=aggT_s[:, ch * 128:(ch + 1) * 128],
                            rhs=wts_s[:, (wbase * kc + ch) * D:
                                      (wbase * kc + ch + 1) * D],
                            start=(ch == 0), stop=False)
                    for ch in range(kc):
                        nc.tensor.matmul(
                            ps[:],
                            lhsT=xt_s[:, ch, :],
                            rhs=wts_s[:, ((wbase + 1) * kc + ch) * D:
                                      ((wbase + 1) * kc + ch + 1) * D],
                            start=False, stop=False)
                    nc.tensor.matmul(
                        ps[:], lhsT=ones_s[:],
                        rhs=bias_s[:, bcol:bcol + D],
                        start=False, stop=True)
                # ---- epilogue + writeback
                loc_s = wpool.tile([128, D], odt, tag="locs")
                nc.vector.tensor_scalar(
                    loc_s[:], ps_loc[:], -100.0, 100.0,
                    mybir.AluOpType.max, mybir.AluOpType.min)
                nc.sync.dma_start(loc_d[b * 128:(b + 1) * 128, :], loc_s[:])
                if cfg.use_softplus:
                    sp_s = wpool.tile([128, D], f32, tag="sps")
                    nc.scalar.activation(
                        sp_s[:], ps_scl[:],
                        mybir.ActivationFunctionType.Softplus)
                else:
                    # softplus(z) = ln(exp(z) + 1); overflow to inf is
                    # absorbed by min(., 100) since softplus(z)≈z there
                    ex_s = wpool.tile([128, D], f32, tag="exs")
                    nc.scalar.activation(
                        ex_s[:], ps_scl[:], mybir.ActivationFunctionType.Exp)
                    sp_s = wpool.tile([128, D], f32, tag="sps")
                    nc.scalar.activation(
                        sp_s[:], ex_s[:], mybir.ActivationFunctionType.Ln,
                        bias=1.0)
                scl_s = wpool.tile([128, D], odt, tag="scls")
                nc.vector.tensor_scalar(
                    scl_s[:], sp_s[:], 0.001, 100.0,
                    mybir.AluOpType.add, mybir.AluOpType.min)
                nc.sync.dma_start(scale_d[b * 128:(b + 1) * 128, :], scl_s[:])

    nc.compile()
    return nc


# ---------------------------------------------------------------- driver

_CACHE = {}


def _get_program(cfg, caps, total_tiles):
    key = (cfg.N, cfg.E, cfg.D, cfg.C, cfg.gather_bf16, cfg.out_bf16,
           cfg.gather_chunk_tiles, cfg.nan_safe_pad, cfg.gbufs,
           cfg.act_copies, cfg.use_softplus, cfg.n_swdge_queues,
           caps.tobytes())
    if key not in _CACHE:
        _CACHE[key] = _build_program(cfg, caps, total_tiles)
    return _CACHE[key]


def _run_on_hw(nc, in_maps, cfg):
    from concourse.bass_utils import run_bass_kernel_spmd
    res = run_bass_kernel_spmd(nc, in_maps, core_ids=list(range(cfg.C)))
    return res.results


def _assemble(cfg, results):
    N, D, NPC = cfg.N, cfg.D, cfg.NPC
    loc = np.empty((N, D), np.float32)
    scale = np.empty((N, D), np.float32)
    for c in range(cfg.C):
        loc[c * NPC:(c + 1) * NPC] = results[c]["loc"][:NPC].astype(
            np.float32)
        scale[c * NPC:(c + 1) * NPC] = results[c]["scale"][:NPC].astype(
            np.float32)
    return loc, scale


def run(x, edge_index, Wl1, bl1, Wr1, Wl2, bl2, Wr2, cfg=None):
    cfg = cfg or CFG
    caps, total_tiles, in_maps = _prep_host(
        cfg, x, edge_index, Wl1, bl1, Wr1, Wl2, bl2, Wr2)
    nc = _get_program(cfg, caps, total_tiles)
    results = _run_on_hw(nc, in_maps, cfg)
    return _assemble(cfg, results)


def kernel(x, edge_index, Wl1, bl1, Wr1, Wl2, bl2, Wr2):
    return run(x, edge_index, Wl1, bl1, Wr1, Wl2, bl2, Wr2)


# ---------------------------------------------------------------- bench

def _install_ntff_hook():
    """The agent image's antenv lacks axon_hooks; recreate it so
    run_bass_kernel_spmd(trace=True) can NTFF-profile under axon."""
    import sys
    import types
    if "antenv.axon_hooks" in sys.modules:
        return
    import antenv  # noqa: F401
    mod = types.ModuleType("antenv.axon_hooks")
    state = {"hook": None}
    mod.set_axon_ntff_profile_hook = lambda h: state.update(hook=h)
    mod.get_axon_ntff_profile_hook = lambda: state["hook"]
    sys.modules["antenv.axon_hooks"] = mod
    from trn_agent_boot.trn_boot import _ntff_profile_via_ctypes
    mod.set_axon_ntff_profile_hook(
        _ntff_profile_via_ctypes("/opt/axon/libaxon_pjrt.so"))


def bench_ns(x, edge_index, Wl1, bl1, Wr1, Wl2, bl2, Wr2,
             cfg=None, reps=None):
    """Measure device exec time via NTFF profiling (neuron-profile)."""
    import tempfile
    cfg = cfg or CFG
    _install_ntff_hook()
    caps, total_tiles, in_maps = _prep_host(
        cfg, x, edge_index, Wl1, bl1, Wr1, Wl2, bl2, Wr2)
    nc = _get_program(cfg, caps, total_tiles)
    from concourse.bass_utils import run_bass_kernel_spmd
    tmpdir = tempfile.mkdtemp(prefix="bass_profile_")
    res = run_bass_kernel_spmd(
        nc, in_maps, core_ids=list(range(cfg.C)),
        trace=True, tmpdir=tmpdir, trace_cores=[0])
    trace_path = (res.instructions_and_trace[1]
                  if res.instructions_and_trace else None)
    return res.exec_time_ns, {"trace": trace_path, "tmpdir": tmpdir}


# revision 16
# speedup vs baseline: 1.1658x; 1.1658x over previous
"""Trainium2 Bass kernel for DiagGraphSAGENet (GraphSAGE message passing).

Computes, for node features x [N, 512] and edge list [2, E]:
    agg   = segment_sum(x[src], dst)                      # sum over in-edges
    loc   = clip(agg @ Wl1.T + bl1 + x @ Wr1.T, -100, 100)
    scale = min(softplus(agg @ Wl2.T + bl2 + x @ Wr2.T) + 0.001, 100)

Strategy (8 NeuronCores, SPMD single NEFF):
  - Destination-node sharding: core c owns nodes [c*6250, (c+1)*6250).
  - Host sorts edges by (dst core, 128-node dst block, src-half); device
    gathers source rows with the Q7 dma_gather primitive (int16 indices,
    so x is split at row 32768 into lo/hi gather sources). x is cast to
    bf16 host-side to halve the gather traffic (the dominant DMA term).
  - Segment sum realized as one-hot matmul: for each 128-edge tile, build
    M[edge, dst_local] = (dstloc[edge] == iota) on DVE (bf16), accumulate
    psum_agg[128 nodes, 512] += M.T @ Xe on the tensor engine.
  - agg transposed on-PE; the four 512x512 GEMMs run in bf16 against
    host-pretransposed bf16 weights; biases folded in via a K=1 ones-row
    matmul; clip/softplus epilogue on DVE+ACT (single Softplus table).
  - PSUM->SBUF copies run on the scalar engine to keep DVE free.
  - Each core writes bf16 loc/scale for its 6250 nodes; host reassembles
    and casts to f32.
"""

import math
import numpy as np

# ---------------------------------------------------------------- config

class Cfg:
    def __init__(self, n_nodes=50000, n_edges=800000, d=512, n_cores=8,
                 gather_bf16=True, out_bf16=True, nan_safe_pad=True,
                 gather_chunk_tiles=4, gbufs=4, act_copies=True,
                 use_softplus=False, n_swdge_queues=4):
        self.N = n_nodes
        self.E = n_edges
        self.D = d
        self.C = n_cores
        assert n_nodes % n_cores == 0
        self.NPC = n_nodes // n_cores            # nodes per core
        self.B = math.ceil(self.NPC / 128)       # dst blocks per core
        self.ROWS = self.B * 128                 # padded out rows per core
        self.SPLIT = min(32768, n_nodes)         # int16 gather split point
        self.gather_bf16 = gather_bf16
        self.out_bf16 = out_bf16
        # pad gather slots with idx=0 (real row, avoids NaN garbage in SBUF)
        self.nan_safe_pad = nan_safe_pad
        # max 128-row tiles per dma_gather instruction
        self.gather_chunk_tiles = gather_chunk_tiles
        self.gbufs = gbufs
        self.act_copies = act_copies
        self.use_softplus = use_softplus
        self.n_swdge_queues = n_swdge_queues


CFG = Cfg()

# ---------------------------------------------------------------- host prep

def _prep_edges(cfg, src, dst):
    """Sort/pad edges into per-core per-block gather streams.

    Returns (caps [B,2] shared tile caps, tile_off [B,2], total_tiles,
    idx_grid [C,16,8*total_tiles] int16, dstloc [C,128,total_tiles] f32).
    """
    C, B, NPC, SPLIT = cfg.C, cfg.B, cfg.NPC, cfg.SPLIT
    ecore = dst // NPC
    eblk = (dst % NPC) // 128
    eslot = (dst % NPC) % 128
    ehi = (src >= SPLIT).astype(np.int64)
    key = (ecore * B + eblk) * 2 + ehi
    # secondary sort by src so each segment's gather descriptors sweep HBM
    # monotonically (row-buffer locality on the random gather)
    order = np.lexsort((src, key))
    src_s = src[order]
    eslot_s = eslot[order]
    counts = np.bincount(key, minlength=C * B * 2)
    start = np.concatenate([[0], np.cumsum(counts)])
    cnt = counts.reshape(C, B, 2)
    caps = -(-cnt // 128)                # ceil tiles per (core, blk, stream)
    caps = caps.max(axis=0)              # [B, 2] shared across cores (SPMD)
    tile_off = np.zeros((B, 2), np.int64)
    off = 0
    for b in range(B):
        for s in range(2):
            tile_off[b, s] = off
            off += caps[b, s]
    total_tiles = int(off)

    pad_idx = 0 if cfg.nan_safe_pad else -1
    idx_grid = np.full((C, 16, 8 * total_tiles), pad_idx, np.int16)
    dstloc = np.full((C, 128, total_tiles), -1.0, np.float32)
    for c in range(C):
        for b in range(B):
            for s in (0, 1):
                T = int(caps[b, s])
                if T == 0:
                    continue
                k = (c * B + b) * 2 + s
                n = int(counts[k])
                toff = int(tile_off[b, s])
                buf = np.full(T * 128, pad_idx, np.int64)
                buf[:n] = src_s[start[k]:start[k] + n] - (SPLIT if s else 0)
                dl = np.full(T * 128, -1.0, np.float32)
                dl[:n] = eslot_s[start[k]:start[k] + n]
                idx_grid[c, :, 8 * toff:8 * (toff + T)] = (
                    buf.reshape(-1, 16).T.astype(np.int16))
                dstloc[c, :, toff:toff + T] = dl.reshape(T, 128).T
    return caps, tile_off, total_tiles, idx_grid, dstloc


def _prep_host(cfg, x, edge_index, Wl1, bl1, Wr1, Wl2, bl2, Wr2):
    import ml_dtypes
    bf16 = ml_dtypes.bfloat16
    src = np.asarray(edge_index[0]).astype(np.int64)
    dst = np.asarray(edge_index[1]).astype(np.int64)
    x = np.asarray(x, dtype=np.float32)
    caps, tile_off, total_tiles, idx_grid, dstloc = _prep_edges(cfg, src, dst)

    # per-core transposed own features, padded to ROWS columns (bf16)
    xt = np.zeros((cfg.C, cfg.D, cfg.ROWS), bf16)
    for c in range(cfg.C):
        own = x[c * cfg.NPC:(c + 1) * cfg.NPC]
        xt[c][:, :cfg.NPC] = own.T.astype(bf16)

    # weights packed as [128, 16*D]: for w in (Wl1, Wr1, Wl2, Wr2), chunks
    # c of W.T: rows c*128..c*128+127 -> columns (w*4+c)*D .. +D
    kc = cfg.D // 128
    packs = []
    for W in (Wl1, Wr1, Wl2, Wr2):
        WT = np.asarray(W, np.float32).T                     # [D_in, D_out]
        packs.append(WT.reshape(kc, 128, cfg.D).transpose(1, 0, 2)
                     .reshape(128, kc * cfg.D))
    wts = np.concatenate(packs, axis=1).astype(bf16)         # [128, 4*kc*D]

    bias = np.concatenate([np.asarray(bl1, np.float32),
                           np.asarray(bl2, np.float32)])[None, :].astype(bf16)
    iota = np.tile(np.arange(128, dtype=np.float32),
                   (128, 1)).astype(bf16)                    # [128,128]
    iden = np.eye(128, dtype=np.float32).astype(bf16)
    ones = np.ones((1, 128), bf16)

    xg = x.astype(bf16) if cfg.gather_bf16 else x
    dl16 = dstloc.astype(bf16)
    in_maps = []
    for c in range(cfg.C):
        in_maps.append({
            "x": xg,
            "idx": np.tile(idx_grid[c], (8, 1)),
            "dstloc": dl16[c],
            "xt": xt[c],
            "wts": wts,
            "bias": bias,
            "iota": iota,
            "iden": iden,
            "ones": ones,
        })
    return caps, total_tiles, in_maps


# ---------------------------------------------------------------- device

def _build_program(cfg, caps, total_tiles):
    import concourse.bacc as bacc
    import concourse.mybir as mybir
    import concourse.tile as tile

    f32 = mybir.dt.float32
    bf16 = mybir.dt.bfloat16
    gdt = bf16 if cfg.gather_bf16 else mybir.dt.float32r
    odt = bf16 if cfg.out_bf16 else f32
    D, B, SPLIT, N = cfg.D, cfg.B, cfg.SPLIT, cfg.N
    kc = D // 128

    nc = bacc.Bacc("TRN2", target_bir_lowering=False, debug=False,
                   num_swdge_queues=cfg.n_swdge_queues,
                   dynamic_dma_scratch_size=32768)
    x_d = nc.dram_tensor("x", [N, D], gdt, kind="ExternalInput")
    idx_d = nc.dram_tensor("idx", [128, 8 * total_tiles], mybir.dt.int16,
                           kind="ExternalInput")
    dstloc_d = nc.dram_tensor("dstloc", [128, total_tiles], bf16,
                              kind="ExternalInput")
    xt_d = nc.dram_tensor("xt", [D, cfg.ROWS], bf16, kind="ExternalInput")
    wts_d = nc.dram_tensor("wts", [128, 4 * kc * D], bf16,
                           kind="ExternalInput")
    bias_d = nc.dram_tensor("bias", [1, 2 * D], bf16, kind="ExternalInput")
    iota_d = nc.dram_tensor("iota", [128, 128], bf16, kind="ExternalInput")
    iden_d = nc.dram_tensor("iden", [128, 128], bf16, kind="ExternalInput")
    ones_d = nc.dram_tensor("ones", [1, 128], bf16, kind="ExternalInput")
    loc_d = nc.dram_tensor("loc", [cfg.ROWS, D], odt, kind="ExternalOutput")
    scale_d = nc.dram_tensor("scale", [cfg.ROWS, D], odt,
                             kind="ExternalOutput")

    Tmax = int((caps[:, 0] + caps[:, 1]).max())

    with tile.TileContext(nc) as tc:
        with (
            tc.tile_pool(name="const", bufs=1) as constp,
            tc.tile_pool(name="gbuf", bufs=cfg.gbufs) as gpool,
            tc.tile_pool(name="work", bufs=3) as wpool,
            tc.tile_pool(name="mbuf", bufs=4) as mpool,
            tc.tile_pool(name="psum", bufs=2, space="PSUM") as pp,
        ):
            idx_s = constp.tile([128, 8 * total_tiles], mybir.dt.int16)
            nc.sync.dma_start(idx_s[:], idx_d[:])
            dstloc_s = constp.tile([128, total_tiles], bf16)
            nc.sync.dma_start(dstloc_s[:], dstloc_d[:])
            wts_s = constp.tile([128, 4 * kc * D], bf16)
            nc.sync.dma_start(wts_s[:], wts_d[:])
            bias_s = constp.tile([1, 2 * D], bf16)
            nc.sync.dma_start(bias_s[:], bias_d[:])
            iota_s = constp.tile([128, 128], bf16)
            nc.sync.dma_start(iota_s[:], iota_d[:])
            ident_s = constp.tile([128, 128], bf16)
            nc.sync.dma_start(ident_s[:], iden_d[:])
            ones_s = constp.tile([1, 128], bf16)
            nc.sync.dma_start(ones_s[:], ones_d[:])

            if not cfg.nan_safe_pad:
                # pad slots use idx=-1: the gather ucode trims trailing
                # negative idxs, leaving stale SBUF data in those slots.
                # M zeroes their contribution, but 0*NaN=NaN -- so zero the
                # gather ring slots once to guarantee finite stale data.
                for _ in range(cfg.gbufs):
                    gx0 = gpool.tile([128, Tmax * D], gdt, tag="gx")
                    nc.vector.memset(gx0[:], 0.0)

            gq = 0  # round-robin SWDGE queue -> Q7 core pair (2q, 2q+1)
            for b in range(B):
                Tlo, Thi = int(caps[b, 0]), int(caps[b, 1])
                Tb = Tlo + Thi
                toff = int(np.sum(caps[:b]))  # tiles before block b
                # ---- gather source rows for this block's edges
                if Tb > 0:
                    gx = gpool.tile([128, Tmax * D], gdt, tag="gx")
                    GC = cfg.gather_chunk_tiles
                    for seg_T, seg_src, seg_t0, dst_t0 in (
                            (Tlo, x_d[0:SPLIT, :], toff, 0),
                            (Thi, x_d[SPLIT:N, :], toff + Tlo, Tlo)):
                        for t0 in range(0, seg_T, GC):
                            tn = min(GC, seg_T - t0)
                            nc.gpsimd.dma_gather(
                                out_ap=gx[:, (dst_t0 + t0) * D:
                                          (dst_t0 + t0 + tn) * D].rearrange(
                                    "p (t e) -> p t e", e=D),
                                in_ap=seg_src,
                                idxs_ap=idx_s[:, 8 * (seg_t0 + t0):
                                              8 * (seg_t0 + t0 + tn)],
                                num_idxs=tn * 128, num_idxs_reg=tn * 128,
                                elem_size=D,
                                queue_num=gq % cfg.n_swdge_queues)
                            gq += 1
                # ---- own features (transposed) for this block
                xt_s = wpool.tile([128, kc, 128], bf16, tag="xt")
                nc.sync.dma_start(
                    xt_s[:],
                    xt_d[:, b * 128:(b + 1) * 128].rearrange(
                        "(c p) n -> p c n", p=128))
                # ---- aggregation: psum_agg[node, feat] += M.T @ Xe
                agg_s = wpool.tile([128, D], bf16, tag="aggs")
                if Tb > 0:
                    ps_agg = pp.tile([128, D], f32, tag="agg")
                    for t in range(Tb):
                        m = mpool.tile([128, 128], gdt, tag="m")
                        nc.vector.tensor_tensor(
                            out=m[:],
                            in0=dstloc_s[:, toff + t:toff + t + 1]
                                .to_broadcast([128, 128]),
                            in1=iota_s[:],
                            op=mybir.AluOpType.is_equal)
                        nc.tensor.matmul(
                            ps_agg[:], lhsT=m[:],
                            rhs=gx[:, t * D:(t + 1) * D],
                            start=(t == 0), stop=(t == Tb - 1))
                    if cfg.act_copies:
                        nc.scalar.activation(
                            agg_s[:], ps_agg[:],
                            mybir.ActivationFunctionType.Copy)
                    else:
                        nc.vector.tensor_copy(agg_s[:], ps_agg[:])
                else:
                    nc.vector.memset(agg_s[:], 0.0)
                # ---- transpose agg -> aggT (feat-major for GEMM lhsT)
                ps_t = pp.tile([128, D], bf16, tag="aggT")
                for ch in range(kc):
                    nc.tensor.transpose(
                        ps_t[:, ch * 128:(ch + 1) * 128],
                        agg_s[:, ch * 128:(ch + 1) * 128],
                        ident_s[:])
                aggT_s = wpool.tile([128, D], bf16, tag="aggTs")
                if cfg.act_copies:
                    nc.scalar.activation(
                        aggT_s[:], ps_t[:],
                        mybir.ActivationFunctionType.Copy)
                else:
                    nc.vector.tensor_copy(aggT_s[:], ps_t[:])
                # ---- GEMMs: loc / scale heads
                ps_loc = pp.tile([128, D], f32, tag="loc")
                ps_scl = pp.tile([128, D], f32, tag="scl")
                for ps, wbase, bcol in ((ps_loc, 0, 0), (ps_scl, 2, D)):
                    for ch in range(kc):
                        nc.tensor.matmul(
                            ps[:],
                            lhsT=aggT_s[:, ch * 128:(ch + 1) * 128],
                            rhs=wts_s[:, (wbase * kc + ch) * D:
                                      (wbase * kc + ch + 1) * D],
                            start=(ch == 0), stop=False)
                    for ch in range(kc):
                        nc.tensor.matmul(
                            ps[:],
                            lhsT=xt_s[:, ch, :],
                            rhs=wts_s[:, ((wbase + 1) * kc + ch) * D:
                                      ((wbase + 1) * kc + ch + 1) * D],
                            start=False, stop=False)
                    nc.tensor.matmul(
                        ps[:], lhsT=ones_s[:],
                        rhs=bias_s[:, bcol:bcol + D],
                        start=False, stop=True)
                # ---- epilogue + writeback
                loc_s = wpool.tile([128, D], odt, tag="locs")
                nc.vector.tensor_scalar(
                    loc_s[:], ps_loc[:], -100.0, 100.0,
                    mybir.AluOpType.max, mybir.AluOpType.min)
                nc.sync.dma_start(loc_d[b * 128:(b + 1) * 128, :], loc_s[:])
                if cfg.use_softplus:
                    sp_s = wpool.tile([128, D], f32, tag="sps")
                    nc.scalar.activation(
                        sp_s[:], ps_scl[:],
                        mybir.ActivationFunctionType.Softplus)
                else:
                    # softplus(z) = ln(exp(z) + 1); overflow to inf is
                    # absorbed by min(., 100) since softplus(z)≈z there
                    ex_s = wpool.tile([128, D], f32, tag="exs")
                    nc.scalar.activation(
                        ex_s[:], ps_scl[:], mybir.ActivationFunctionType.Exp)
                    sp_s = wpool.tile([128, D], f32, tag="sps")
                    nc.scalar.activation(
                        sp_s[:], ex_s[:], mybir.ActivationFunctionType.Ln,
                        bias=1.0)
                scl_s = wpool.tile([128, D], odt, tag="scls")
                nc.vector.tensor_scalar(
                    scl_s[:], sp_s[:], 0.001, 100.0,
                    mybir.AluOpType.add, mybir.AluOpType.min)
                nc.sync.dma_start(scale_d[b * 128:(b + 1) * 128, :], scl_s[:])

    nc.compile()
    return nc


# ---------------------------------------------------------------- driver

_CACHE = {}


def _get_program(cfg, caps, total_tiles):
    key = (cfg.N, cfg.E, cfg.D, cfg.C, cfg.gather_bf16, cfg.out_bf16,
           cfg.gather_chunk_tiles, cfg.nan_safe_pad, cfg.gbufs,
           cfg.act_copies, cfg.use_softplus, cfg.n_swdge_queues,
           caps.tobytes())
    if key not in _CACHE:
        _CACHE[key] = _build_program(cfg, caps, total_tiles)
    return _CACHE[key]


def _run_on_hw(nc, in_maps, cfg):
    from concourse.bass_utils import run_bass_kernel_spmd
    res = run_bass_kernel_spmd(nc, in_maps, core_ids=list(range(cfg.C)))
    return res.results


def _assemble(cfg, results):
    N, D, NPC = cfg.N, cfg.D, cfg.NPC
    loc = np.empty((N, D), np.float32)
    scale = np.empty((N, D), np.float32)
    for c in range(cfg.C):
        loc[c * NPC:(c + 1) * NPC] = results[c]["loc"][:NPC].astype(
            np.float32)
        scale[c * NPC:(c + 1) * NPC] = results[c]["scale"][:NPC].astype(
            np.float32)
    return loc, scale


def run(x, edge_index, Wl1, bl1, Wr1, Wl2, bl2, Wr2, cfg=None):
    cfg = cfg or CFG
    caps, total_tiles, in_maps = _prep_host(
        cfg, x, edge_index, Wl1, bl1, Wr1, Wl2, bl2, Wr2)
    nc = _get_program(cfg, caps, total_tiles)
    results = _run_on_hw(nc, in_maps, cfg)
    return _assemble(cfg, results)


def kernel(x, edge_index, Wl1, bl1, Wr1, Wl2, bl2, Wr2):
    return run(x, edge_index, Wl1, bl1, Wr1, Wl2, bl2, Wr2)


# ---------------------------------------------------------------- bench

def _install_ntff_hook():
    """The agent image's antenv lacks axon_hooks; recreate it so
    run_bass_kernel_spmd(trace=True) can NTFF-profile under axon."""
    import sys
    import types
    if "antenv.axon_hooks" in sys.modules:
        return
    import antenv  # noqa: F401
    mod = types.ModuleType("antenv.axon_hooks")
    state = {"hook": None}
    mod.set_axon_ntff_profile_hook = lambda h: state.update(hook=h)
    mod.get_axon_ntff_profile_hook = lambda: state["hook"]
    sys.modules["antenv.axon_hooks"] = mod
    from trn_agent_boot.trn_boot import _ntff_profile_via_ctypes
    mod.set_axon_ntff_profile_hook(
        _ntff_profile_via_ctypes("/opt/axon/libaxon_pjrt.so"))


def bench_ns(x, edge_index, Wl1, bl1, Wr1, Wl2, bl2, Wr2,
             cfg=None, reps=None):
    """Measure device exec time via NTFF profiling (neuron-profile)."""
    import tempfile
    cfg = cfg or CFG
    _install_ntff_hook()
    caps, total_tiles, in_maps = _prep_host(
        cfg, x, edge_index, Wl1, bl1, Wr1, Wl2, bl2, Wr2)
    nc = _get_program(cfg, caps, total_tiles)
    from concourse.bass_utils import run_bass_kernel_spmd
    tmpdir = tempfile.mkdtemp(prefix="bass_profile_")
    res = run_bass_kernel_spmd(
        nc, in_maps, core_ids=list(range(cfg.C)),
        trace=True, tmpdir=tmpdir, trace_cores=[0])
    trace_path = (res.instructions_and_trace[1]
                  if res.instructions_and_trace else None)
    return res.exec_time_ns, {"trace": trace_path, "tmpdir": tmpdir}
